# revision 2
# baseline (speedup 1.0000x reference)
"""Trainium2 Bass kernel for nn_MeanConv: sum of 7 box-filter means (k=3..15,
edge padding) averaged and masked by map_f.

Math: out[i,j] = sum_{|a|<=7,|b|<=7} W[a,b] * xpad[i+a, j+b] * map[i,j]
with W[a,b] = (1/7) * sum_{k in {3,5,..,15}, k//2 >= max(|a|,|b|)} 1/k^2.

Device strategy (per core, 512 output rows):
  out_tile[M=114, N=512] = sum_{b=-7..7} A_{|b|}^T @ x_tile[:, +b]
where A_m[r, i] = W[r-i-7, m] is a [K=128, M=114] band matrix (constant).
15 accumulating float32r matmuls per PSUM tile -> 1 cycle/row on PE.
Epilogue: one DVE multiply by map, DMA out. Host does halo sharding
(each core's x slice carries its 7-row halos) so no collectives.
"""

import sys
import time

import numpy as np

sys.path.insert(0, "/opt/trn_rl_repo")

KERNELS = (3, 5, 7, 9, 11, 13, 15)
H = W = 4096
P = 7            # max halo (k_max // 2)
N_CORES = 8
ROWS_PER_CORE = H // N_CORES          # 512
SLICE_ROWS = ROWS_PER_CORE + 2 * P    # 526
PADW = W + 2 * P                      # 4110
STRIPE = 114                          # M per matmul; K = M + 14 = 128
NCHUNK = 512                          # N per matmul (PSUM bank limit, fp32)

_EXEC_NS = [None]


def _weight_matrices():
    """A_m [K=128, M=114] for m=0..7, float32."""
    # w2d[a+7, b+7] = (1/7) * sum over kernels covering offset (a, b)
    w1 = np.zeros(P + 1, dtype=np.float64)   # w1[m] = sum_{k: k//2 >= m} 1/k^2
    for m in range(P + 1):
        w1[m] = sum(1.0 / (k * k) for k in KERNELS if k // 2 >= max(m, 1))
    w1 /= len(KERNELS)
    A = np.zeros((P + 1, 2 * STRIPE // 2 + 14, STRIPE), dtype=np.float64)
    A = np.zeros((P + 1, STRIPE + 14, STRIPE), dtype=np.float64)
    for m in range(P + 1):
        for i in range(STRIPE):
            for a in range(-P, P + 1):
                A[m, i + a + P, i] = w1[max(abs(a), m)]
    return A.astype(np.float32)  # (8, 128, 114)


def _build_bass():
    import concourse.bass as bass  # noqa: F401
    import concourse.mybir as mybir
    import concourse.tile as tile
    from concourse import bacc

    f32 = mybir.dt.float32
    f32r = mybir.dt.float32r

    nc = bacc.Bacc("TRN2", target_bir_lowering=False, debug=False)

    xs = nc.dram_tensor("xs", [SLICE_ROWS, PADW], f32r, kind="ExternalInput").ap()
    mp = nc.dram_tensor("mp", [ROWS_PER_CORE, W], f32, kind="ExternalInput").ap()
    wm = nc.dram_tensor("wm", [P + 1, 128, STRIPE], f32r, kind="ExternalInput").ap()
    out = nc.dram_tensor("out", [ROWS_PER_CORE, W], f32, kind="ExternalOutput").ap()

    stripes = []
    r = 0
    while r < ROWS_PER_CORE:
        m = min(STRIPE, ROWS_PER_CORE - r)
        stripes.append((r, m))
        r += m

    with tile.TileContext(nc) as tc:
        with (
            tc.tile_pool(name="wpool", bufs=1) as wpool,
            tc.tile_pool(name="xpool", bufs=2) as xpool,
            tc.tile_pool(name="mpool", bufs=3) as mpool,
            tc.tile_pool(name="opool", bufs=3) as opool,
            tc.tile_pool(name="psum", bufs=4, space="PSUM") as pspool,
        ):
            wt = []
            for j in range(P + 1):
                t = wpool.tile([128, STRIPE], f32r, tag=f"w{j}")
                nc.sync.dma_start(t[:], wm[j])
                wt.append(t)

            for (r0, m) in stripes:
                k = m + 2 * P  # input rows needed: 128 or 70
                xt = xpool.tile([128, PADW], f32r)
                nc.sync.dma_start(xt[:k, :], xs[r0 : r0 + k, :])
                for c in range(W // NCHUNK):
                    ps = pspool.tile([STRIPE, NCHUNK], f32)
                    for b in range(-P, P + 1):
                        nc.tensor.matmul(
                            ps[:m, :],
                            lhsT=wt[abs(b)][:k, :m],
                            rhs=xt[:k, c * NCHUNK + P + b : c * NCHUNK + P + b + NCHUNK],
                            start=(b == -P),
                            stop=(b == P),
                        )
                    mt = mpool.tile([STRIPE, NCHUNK], f32)
                    nc.sync.dma_start(
                        mt[:m, :], mp[r0 : r0 + m, c * NCHUNK : (c + 1) * NCHUNK]
                    )
                    ot = opool.tile([STRIPE, NCHUNK], f32)
                    nc.vector.tensor_mul(ot[:m, :], ps[:m, :], mt[:m, :])
                    nc.sync.dma_start(
                        out[r0 : r0 + m, c * NCHUNK : (c + 1) * NCHUNK], ot[:m, :]
                    )
    nc.compile()
    return nc


_NC_CACHE = [None]


def kernel(x: np.ndarray, map_f: np.ndarray) -> np.ndarray:
    from concourse.bass_utils import run_bass_kernel_spmd

    xsq = np.ascontiguousarray(x[0, 0], dtype=np.float32)
    xp = np.pad(xsq, P, mode="edge")  # (4110, 4110)
    wmat = _weight_matrices()

    in_maps = []
    for c in range(N_CORES):
        r0 = c * ROWS_PER_CORE
        in_maps.append(
            {
                "xs": np.ascontiguousarray(xp[r0 : r0 + SLICE_ROWS, :]),
                "mp": np.ascontiguousarray(map_f[r0 : r0 + ROWS_PER_CORE, :]),
                "wm": wmat,
            }
        )

    if _NC_CACHE[0] is None:
        _NC_CACHE[0] = _build_bass()
    nc = _NC_CACHE[0]

    res = run_bass_kernel_spmd(nc, in_maps, core_ids=list(range(N_CORES)))
    t0 = time.perf_counter()
    res = run_bass_kernel_spmd(nc, in_maps, core_ids=list(range(N_CORES)))
    _EXEC_NS[0] = (time.perf_counter() - t0) * 1e9

    full = np.concatenate([res.results[c]["out"] for c in range(N_CORES)], axis=0)
    return full.reshape(1, 1, H, W).astype(np.float32)
